# revision 2
# baseline (speedup 1.0000x reference)
"""Trainium2 Bass kernel for the BSplineBasis (KAN-style) layer.

Math:
  out[b,o] = sum_{i,k} C[o,i,k]*scale[o]*basis_k(clip(x[b,i])) + sum_i W[o,i]*x[b,i] + bias[o]

Reformulated as ONE matmul with 12 feature planes per input channel i:
  planes 0..10: basis_k(xc) (cubic cardinal B-spline, closed form)
  plane  11  : raw x (residual)
Contraction dim = 12*1024 = 12288. Weights are host-folded:
  Wbig[k*I+i, o] = C[o,i,k]*scale[o] + bias[o]/I   (partition of unity folds the bias)
  Wbig[11*I+i, o] = W[o,i]

Closed-form basis (uniform knots, h=0.25, s = 4*xc+4 in [0,8]):
  d_k = |s - (k-1)|;  basis_k = relu(2-d_k)^3/6 - (2/3)*relu(1-d_k)^3

Per-core device pipeline (batch-sharded, 512 rows/core), per plane k:
  ACT: d=Abs(4*xc+(5-k)) f32; a1=Relu(2-d) bf16; b1=Relu(c4*(1-d)) bf16
       (c4 = 4^(1/3) so b1^3 = 4*relu(1-d)^3)
  DVE (bf16 2x): a3 = a1^3 via 2 TT; b34 = b1^3 via 2 TT; fk = a3-b34 = 6*basis_k
  The 1/6 is folded into the host weights.
  PE : 96 k-chunks x 8 matmuls accumulate [128b x 512o] fp32 PSUM tiles
"""

import numpy as np
import ml_dtypes

B, I, O, K = 4096, 1024, 1024, 11
NCORES = 8
BS = B // NCORES          # 512 batch rows per core
NPLANES = K + 1           # 12
NF = NPLANES * I          # 12288 feature rows
NCHUNK = NF // 128        # 96
FD = (I // 128) * BS      # 4096 free dim of plane tiles: (i_chunk, b)

_cache = {}


def _build_bass(use_pow=True, use_constact=True, use_strided_dma=True,
                use_matmul=True, loop_n=0, loop_scope="all", stub_planes=False,
                skip_reload=True, no_wdma=False, wbufs=3, w_gpsimd=False):
    import concourse.bass as bass
    import concourse.tile as tile
    from concourse import bacc, mybir
    from contextlib import ExitStack

    F32 = mybir.dt.float32
    BF16 = mybir.dt.bfloat16
    AL = mybir.AluOpType
    AF = mybir.ActivationFunctionType

    nc = bacc.Bacc("TRN2", debug=False, num_devices=NCORES)

    if use_constact:
        # Register const APs for the float biases used by nc.scalar.activation.
        need = {float(v) for v in range(-5, 6)} | {2.0, 4.0 ** (1.0 / 3.0)}
        for v in sorted(need):
            key = (F32, v)
            if key not in nc.const_aps.aps:
                t = nc.alloc_sbuf_tensor(f"constb-{v}", [128, 1], F32)
                nc.gpsimd.memset(t.ap(), v)
                nc.const_aps.aps[key] = t.ap()
        nc.all_engine_barrier()

    xt = nc.dram_tensor("xt", [I, BS], F32, kind="ExternalInput")
    w = nc.dram_tensor("wbig", [NF, O], BF16, kind="ExternalInput")
    out = nc.dram_tensor("out", [BS, O], F32, kind="ExternalOutput")

    HINTS = ()
    with tile.TileContext(nc) as tc, ExitStack() as ctx:
        HINTS = (mybir.EngineType.PE, mybir.EngineType.DVE,
                 mybir.EngineType.Activation, mybir.EngineType.SP)
        if loop_n and loop_scope == "all":
            ctx.enter_context(tc.For_i(0, loop_n, 1, hint_engines=HINTS))
        xpool = ctx.enter_context(tc.tile_pool(name="x", bufs=1))
        fpool = ctx.enter_context(tc.tile_pool(name="f", bufs=4))
        rpool = ctx.enter_context(tc.tile_pool(name="r", bufs=1))
        dpool = ctx.enter_context(tc.tile_pool(name="d", bufs=2))
        tpool = ctx.enter_context(tc.tile_pool(name="t", bufs=1))
        wpool = ctx.enter_context(tc.tile_pool(name="w", bufs=wbufs))
        opool = ctx.enter_context(tc.tile_pool(name="o", bufs=8))
        pspool = ctx.enter_context(tc.tile_pool(name="ps", bufs=1, space="PSUM"))

        # ---- load x transposed: [1024 i, 512 b] -> one [128, 4096] tile ----
        xsb = xpool.tile([128, FD], F32, tag="xsb")
        fres = rpool.tile([128, FD], BF16, tag="fres")
        xc = xpool.tile([128, FD], F32, tag="xc")
        # per-chunk loads so the residual plane (and first matmuls) start
        # as soon as the first 256KB chunk lands; Bacc spills multi-waits.
        for c in range(I // 128):
            sl = slice(c * BS, (c + 1) * BS)
            nc.gpsimd.dma_start(xsb[:, sl], xt[c * 128:(c + 1) * 128, :])
            nc.vector.tensor_copy(fres[:, sl], xsb[:, sl])
            nc.vector.tensor_scalar(xc[:, sl], xsb[:, sl], -1.0, 1.0,
                                    AL.max, AL.min)

        # ---- 11 basis planes ----
        if not use_constact:
            s_all = xpool.tile([128, FD], F32, tag="s_all")
            nc.vector.tensor_scalar(s_all[:], xc[:], 4.0, 4.0, AL.mult, AL.add)

        C4 = 4.0 ** (1.0 / 3.0)
        planes = []
        for k in range(K):
            fk = fpool.tile([128, FD], BF16, tag="fk", name=f"fk{k}")
            if stub_planes:
                nc.vector.memset(fk[:], 0.25)
                planes.append(fk)
                continue
            nsub = 4 if k == 0 else 2
            sw = FD // nsub
            for su in range(nsub):
                sl = slice(su * sw, (su + 1) * sw)
                d = dpool.tile([128, sw], F32, tag="d", bufs=2, name="d")
                nc.scalar.activation(d[:], xc[:, sl], AF.Abs,
                                     bias=float(5 - k), scale=4.0)
                a1 = dpool.tile([128, sw], BF16, tag="a1", bufs=2, name="a1")
                nc.scalar.activation(a1[:], d[:], AF.Relu, bias=2.0, scale=-1.0)
                b1 = dpool.tile([128, sw], BF16, tag="b1", bufs=2, name="b1")
                nc.scalar.activation(b1[:], d[:], AF.Relu, bias=C4, scale=-C4)
                a2 = tpool.tile([128, sw], BF16, tag="a2", bufs=2, name="a2")
                nc.vector.tensor_tensor(a2[:], a1[:], a1[:], AL.mult)
                a3 = tpool.tile([128, sw], BF16, tag="a3", bufs=2, name="a3")
                nc.vector.tensor_tensor(a3[:], a2[:], a1[:], AL.mult)
                b2 = tpool.tile([128, sw], BF16, tag="b2", bufs=2, name="b2")
                nc.vector.tensor_tensor(b2[:], b1[:], b1[:], AL.mult)
                b34 = tpool.tile([128, sw], BF16, tag="b34", bufs=2, name="b34")
                nc.vector.tensor_tensor(b34[:], b2[:], b1[:], AL.mult)
                nc.vector.tensor_tensor(fk[:, sl], a3[:], b34[:], AL.subtract)
            planes.append(fk)
        planes.append(fres)

        if loop_n and loop_scope == "mm":
            ctx.enter_context(tc.For_i(0, loop_n, 1, hint_engines=HINTS))
        # ---- matmul: accumulate [128 b x 512 o] x (4 bc x 2 oh) = 8 PSUM banks
        # Residual plane (ready right after the x DMA) goes FIRST so the PE
        # warms up while the basis planes are still being produced.
        ps = [pspool.tile([128, 512], F32, name=f"ps{j}", tag=f"ps{j}")
              for j in range(8)]
        forder = list(range(11 * 8, NCHUNK)) + list(range(11 * 8))
        wt0 = None
        for pos, f in enumerate(forder):
            k, c = divmod(f, I // 128)
            if no_wdma:
                if wt0 is None:
                    wt0 = wpool.tile([128, O], BF16, tag="wt")
                    nc.sync.dma_start(wt0[:], w[f * 128:(f + 1) * 128, :])
                wt = wt0
            else:
                wt = wpool.tile([128, O], BF16, tag="wt")
                weng = nc.gpsimd if w_gpsimd else nc.sync
                weng.dma_start(wt[:], w[f * 128:(f + 1) * 128, :])
            src = planes[k]
            for bc in range(4):
                lhsT = src[:, c * BS + bc * 128: c * BS + (bc + 1) * 128]
                for oh in range(2):
                    nc.tensor.matmul(ps[bc * 2 + oh][:], lhsT,
                                     wt[:, oh * 512:(oh + 1) * 512],
                                     start=(pos == 0),
                                     stop=(pos == NCHUNK - 1))

        # ---- epilogue: per-bank PSUM -> SBUF -> HBM, engines alternated ----
        for bc in range(4):
            for oh in range(2):
                obh = opool.tile([128, 512], F32, tag="ob", name=f"ob{bc}{oh}")
                if oh == 0:
                    nc.scalar.copy(obh[:], ps[bc * 2 + oh][:])
                else:
                    nc.vector.tensor_copy(obh[:], ps[bc * 2 + oh][:])
                nc.sync.dma_start(
                    out[bc * 128:(bc + 1) * 128, oh * 512:(oh + 1) * 512],
                    obh[:])

    nc.compile()
    if skip_reload:
        _dedupe_ldweights(nc, mybir)
    return nc


def _dedupe_ldweights(nc, mybir):
    """Drop an Ldweights that reloads the exact same weights as the previous
    Ldweights on the PE stream with only Matmults in between (the oh=0/oh=1
    pair shares its stationary operand). The duplicate carries no sync here;
    bail on any with sync_info."""
    import json as _json
    for fn in nc.m.functions:
        for blk in fn.blocks:
            insts = list(blk.instructions)
            kept = []
            last_key = None
            removed = 0
            for inst in insts:
                if inst.engine != mybir.EngineType.PE:
                    kept.append(inst)
                    continue
                op = type(inst).__name__
                if op == "InstLdweights":
                    si = inst.sync_info
                    has_sync = bool(si and (si.on_wait or si.on_update))
                    key = _json.dumps(
                        _json.loads(mybir.instruction_to_pretty_json_string(inst))
                        .get("ins"), sort_keys=True)
                    if key == last_key and not has_sync:
                        removed += 1
                        continue
                    last_key = key
                    kept.append(inst)
                elif op == "InstMatmult":
                    kept.append(inst)
                else:
                    last_key = None
                    kept.append(inst)
            if removed:
                blk.instructions = kept
    return nc


def _fold_weights(spline_coeffs, residual_weight, residual_bias, scale_base):
    scale = scale_base.astype(np.float32).mean(axis=1)              # [O]
    Ws = spline_coeffs.astype(np.float32) * scale[:, None, None]    # [O,I,K]
    Ws = np.ascontiguousarray(Ws.transpose(2, 1, 0))                # [K,I,O]
    Ws += residual_bias.astype(np.float32)[None, None, :] / I
    Ws /= 6.0  # device planes are 6*basis_k
    Wbig = np.concatenate(
        [Ws.reshape(K * I, O),
         np.ascontiguousarray(residual_weight.astype(np.float32).T)], axis=0)
    return np.ascontiguousarray(Wbig.astype(ml_dtypes.bfloat16))    # [NF, O]


def _make_in_maps(inputs):
    wbig = _fold_weights(inputs["spline_coeffs"], inputs["residual_weight"],
                         inputs["residual_bias"], inputs["scale_base"])
    x = np.asarray(inputs["x"], dtype=np.float32)
    in_maps = []
    for c in range(NCORES):
        xs = np.ascontiguousarray(x[c * BS:(c + 1) * BS, :].T)  # [I, BS]
        in_maps.append({"xt": xs, "wbig": wbig})
    return in_maps


def kernel(x, spline_coeffs, residual_weight, residual_bias, scale_base):
    from concourse.bass_utils import run_bass_kernel_spmd

    if "nc" not in _cache:
        _cache["nc"] = _build_bass()
    nc = _cache["nc"]

    in_maps = _make_in_maps(dict(x=x, spline_coeffs=spline_coeffs,
                                 residual_weight=residual_weight,
                                 residual_bias=residual_bias,
                                 scale_base=scale_base))
    res = run_bass_kernel_spmd(nc, in_maps, core_ids=list(range(NCORES)))
    out = np.concatenate([r["out"] for r in res.results], axis=0)
    return out.astype(np.float32)

